# revision 11
# baseline (speedup 1.0000x reference)
"""DefocusLKPN Trainium2 kernel.

Computes, per batch element (reference semantics):
    r      = clip(alpha * defocus + tanh(unet[:,100]), 0, 3)
    disk_k = sigmoid(5*(r - dist_k))            (25 taps, 6 distinct dists)
    w_ck   = exp(l_ck) * disk_k                 (l = unet[:, :100] logits)
    out_c  = sum_k w_ck * patch_ck / sum_k w_ck + x_c

The softmax normalizer and the EPS clamp of the reference cancel exactly
(center tap's disk mask is >= 0.5 for logits of this scale).  The alpha *
defocus product is folded into the defocus array on the host (alpha is a
learned scalar).

Sharding: pure data parallel, batch 16 -> 2 per core across 8 cores.

Per-core layout: partition dim = H (128); free dim packs (b, w) = 256 for
pixel planes and (k, b, w) for the 25-tap weight planes.  The 5x5 unfold is
realized as 5 row-shifted, column-padded copies of x in SBUF (vertical halo)
plus free-dim offsets (horizontal halo); the k-reduction runs on the tensor
engine as identity-matmul accumulation into PSUM (fp16 operands, f32
accumulation).

Performance notes (from HW traces):
  * SBUF->SBUF DMA row streams run at ~17 GB/s serialized on one queue --
    never used here.  The row-shifted x copies are built by shifted-identity
    matmuls into PSUM (tensor engine; zero-fills edge rows) and copied back
    to padded SBUF fp16 tiles by the ACT engine in its idle window between
    the sigmoids and the first exp (gpsimd cannot touch PSUM).
  * DVE fp16 tensor_tensor runs at 0.52 ns/elem (2x mode) with a ~150 ns
    fixed cost per instruction, so the tap-weight products are emitted as
    one 1280-elem instruction per (c, dy) group: mdy tiles pack the 5 m
    planes contiguous then the 5 w planes contiguous; the accumulate matmul
    reads tap j as the two-chunk AP [m_j | w_j].
  * The 25-plane replicated disk mask s25 lets the w-product be a single
    contiguous instruction; built once on the DVE right after the sigmoids.
  * 1/den uses ACT Ln then Exp(scale=-1) (~1.0us/channel) instead of DVE
    InstReciprocal (1.75us/channel).  Epilogue ACT ops are issued one
    channel late (behind the next channel's exps) and epilogue DVE ops after
    the next channel's tap products, so neither ever stalls its engine
    queue; stores are issued on sync two channels late for the same reason.
  * DMA descriptor generation runs at ~1.5 ns per 512B row, so one queue
    tops out near the ~286 GB/s aggregate engine rate.  Loads are split
    {0-9}+{20-24} on sync and {10-19} on gpsimd; the scalar engine issues
    no DMA so exp never stalls a load queue.  Radius-chain elementwise ops
    and the x cast run on gpsimd to keep the vector engine on tap products.
  * The accumulate matmuls are issued as one 20-matmul block (dy 0..3) plus
    the dy4 group, giving the PE long continuous runs to ramp out of the
    low p-state while keeping the post-last-byte tail short.
"""

import sys

sys.path.insert(0, "/opt/trn_rl_repo")

import numpy as np

import concourse.bass as bass
import concourse.mybir as mybir
from concourse.tile import TileContext
from concourse.bass_utils import run_bass_kernel_spmd

F32 = mybir.dt.float32
FP16 = mybir.dt.float16
AF = mybir.ActivationFunctionType
ALU = mybir.AluOpType

MM_DT = FP16

N_CORES = 8
B, C, H, W = 16, 4, 128, 128
BL = B // N_CORES            # 2 batch elements per core
BLC = BL * C                 # 8 (b, c) blocks
KK = 25
BW = BL * W                  # 256: (b, w) free block
WP = W + 4                   # 132: padded width per (b, c) block
DB = 5 * BW                  # 1280: one dy-group block (5 planes)

# distinct tap distances; k = (dy+2)*5 + (dx+2)
DISTS = [0.0, 1.0, np.sqrt(2.0), 2.0, np.sqrt(5.0), np.sqrt(8.0)]
# (dist_index, base_k, [(step, count), (step2, count2)]): tap sets sharing
# that dist, {base + i*s1 + j*s2}.
GROUPS = [
    (0, 12, []),                    # dist 0:      {12}
    (1, 7, [(6, 2), (4, 2)]),       # dist 1:      {7, 11, 13, 17}
    (2, 6, [(10, 2), (2, 2)]),      # dist sqrt2:  {6, 8, 16, 18}
    (3, 2, [(12, 2), (8, 2)]),      # dist 2:      {2, 10, 14, 22}
    (4, 5, [(10, 2), (4, 2)]),      # dist sqrt5:  {5, 9, 15, 19}
    (4, 1, [(20, 2), (2, 2)]),      # dist sqrt5:  {1, 3, 21, 23}
    (5, 0, [(20, 2), (4, 2)]),      # dist sqrt8:  {0, 4, 20, 24}
]

# l-load / exp k-range split: (k0, nk) per slice
KSPLIT = ((0, 10), (10, 10), (20, 5))


def _split_wide_waits(nc, max_waits=1):
    """The walrus build here accepts at most one semaphore wait per
    instruction; move extra waits onto preceding Drains on the same engine."""
    n = 0
    for func in nc.m.functions:
        for bb in func.blocks:
            out = []
            changed = False
            for ins in bb.instructions:
                si = ins.sync_info
                if si is not None and si.on_wait and len(si.on_wait) > max_waits:
                    waits = list(si.on_wait)
                    keep, rest = waits[:max_waits], waits[max_waits:]
                    for i in range(0, len(rest), max_waits):
                        n += 1
                        out.append(
                            mybir.InstDrain(
                                name=f"splitwait-{n}",
                                opcode="Drain",
                                engine=ins.engine,
                                sync_info=mybir.SyncInfo(
                                    on_wait=list(rest[i : i + max_waits]),
                                    on_update=[],
                                ),
                            )
                        )
                    si.on_wait = keep
                    changed = True
                out.append(ins)
            if changed:
                bb.instructions = out
    return n


def _ap(t, extra_off, dims):
    """AP over tile `t` keeping its partition dim, with free dims
    [[step, count], ...] in elements and an extra element offset."""
    return bass.AP(t.tensor, t.offset + extra_off, [list(t.ap[0])] + [list(d) for d in dims])


def _build():
    nc = bass.Bass("TRN2", num_devices=N_CORES)

    xl = nc.dram_tensor("x", [BL, C, H, W], F32, kind="ExternalInput")
    dfl = nc.dram_tensor("defocus", [BL, 1, H, W], F32, kind="ExternalInput")
    ul = nc.dram_tensor("unet", [BL, 4 * KK + 1, H, W], F32, kind="ExternalInput")
    yl = nc.dram_tensor("y", [BL, C, H, W], F32, kind="ExternalOutput")

    # 5 row-shift matrices S_dyi (dyi=0..4 <-> Dy=dyi-2); S.T @ x gives
    # x(i+Dy) with zero fill at the out-of-range edge rows.  Block dyi=2 is
    # the plain identity, reused as the accumulate-matmul stationary.
    s_np = np.zeros((128, 5 * 128), dtype=mybir.dt.np(MM_DT))
    for dyi in range(5):
        s_np[:, dyi * 128 : (dyi + 1) * 128] = np.eye(128, k=2 - dyi)
    sid_dram = nc.inline_tensor(s_np, name="sident")

    UCH = ul.shape[1]          # 101
    HWr = H * W                # plane stride in DRAM

    def load_l(eng, l, c, k0, nk):
        for b in range(BL):
            eng.dma_start(
                out=_ap(l, k0 * BW + b * W, [[BW, nk], [1, W]]),
                in_=bass.AP(
                    ul, (c * KK + k0 + b * UCH) * HWr,
                    [[W, H], [HWr, nk], [1, W]],
                ),
            )

    with TileContext(nc) as tc:
        with (
            tc.tile_pool(name="fix", bufs=1) as fix,
            tc.tile_pool(name="lp", bufs=3) as lp,
            tc.tile_pool(name="ep", bufs=2) as ep,
            tc.tile_pool(name="mp", bufs=6) as mp,
            tc.tile_pool(name="op", bufs=3) as op,
            tc.tile_pool(name="ps", bufs=1, space="PSUM") as ps,
            tc.tile_pool(name="psx", bufs=2, space="PSUM") as psx,
        ):
            # ---- prologue loads (sync queue, radius chain first) ---------
            df = fix.tile([128, BW], F32)
            nc.sync.dma_start(
                out=df[:],
                in_=bass.AP(dfl, 0, [[W, H], [HWr, BL], [1, W]]),
            )
            u100 = fix.tile([128, BW], F32)
            nc.sync.dma_start(
                out=u100[:],
                in_=bass.AP(ul, 100 * HWr, [[W, H], [UCH * HWr, BL], [1, W]]),
            )
            sid = fix.tile([128, 5 * 128], MM_DT)
            nc.gpsimd.dma_start(out=sid[:], in_=sid_dram[:])
            idt = sid[:, 2 * 128 : 3 * 128]
            xf = fix.tile([128, BLC * W], F32)
            nc.gpsimd.dma_start(
                out=xf[:],
                in_=bass.AP(xl, 0, [[W, H], [HWr, BLC], [1, W]]),
            )
            # first channel, finest granularity: 5-plane dy-aligned slices
            # split across both queues so the first tap products start as
            # early as possible (pipeline fill).
            l0 = lp.tile([128, KK * BW], F32, name="l")
            load_l(nc.sync, l0, 0, 0, 5)
            load_l(nc.sync, l0, 0, 5, 5)
            load_l(nc.sync, l0, 0, 20, 5)
            load_l(nc.gpsimd, l0, 0, 10, 5)
            load_l(nc.gpsimd, l0, 0, 15, 5)

            # radius = clip(adf + tanh(u100), 0, 3), adf = alpha*defocus
            # (host-folded).  Elementwise ops on gpsimd: vector stays free.
            dtan = fix.tile([128, BW], F32)
            nc.scalar.activation(dtan[:], u100[:], AF.Tanh)
            r0 = fix.tile([128, BW], F32)
            nc.gpsimd.tensor_tensor(r0[:], df[:], dtan[:], ALU.add)
            rr = fix.tile([128, BW], F32)
            nc.gpsimd.tensor_scalar(rr[:], r0[:], 0.0, 3.0, ALU.max, ALU.min)

            # s6[d] = sigmoid(5*r - 5*dist_d)   (6 planes, shared by all c)
            bt = fix.tile([128, 6], F32)
            for d in range(6):
                nc.gpsimd.memset(bt[:, d : d + 1], float(-5.0 * DISTS[d]))
            s6 = fix.tile([128, 6 * BW], MM_DT)
            for d in range(6):
                nc.scalar.activation(
                    s6[:, d * BW : (d + 1) * BW], rr[:], AF.Sigmoid,
                    bias=bt[:, d : d + 1], scale=5.0,
                )
            # s25[k] = s6[dist(k)]: replicated so the per-(c,dy) w-product is
            # a single contiguous DVE instruction.
            s25 = fix.tile([128, KK * BW], MM_DT)
            for d, base, steps in GROUPS:
                if steps:
                    (s1, c1), (s2, c2) = steps
                    odims = [[s1 * BW, c1], [s2 * BW, c2], [1, BW]]
                    idims = [[0, c1], [0, c2], [1, BW]]
                else:
                    odims = [[1, BW]]
                    idims = [[1, BW]]
                nc.vector.tensor_copy(
                    _ap(s25, base * BW, odims), _ap(s6, d * BW, idims)
                )

            # x cast to the matmul dtype (gpsimd; vector stays free)
            xb = fix.tile([128, BLC * W], MM_DT)
            nc.gpsimd.tensor_copy(xb[:], xf[:])

            # xs[dyi]: row-shifted (by Dy=dyi-2), column-padded (pad 2) fp16
            # copies of x.  Row shift via S_dyi.T @ xb on the tensor engine
            # (PSUM, zero edge rows), copied to SBUF by the ACT engine in
            # its idle window before the first exp.
            xs = []
            for dyi in range(5):
                t = fix.tile([128, BLC * WP], MM_DT, name=f"xs{dyi}")
                nc.gpsimd.memset(_ap(t, 0, [[WP, BLC], [1, 2]]), 0.0)
                nc.gpsimd.memset(_ap(t, 2 + W, [[WP, BLC], [1, 2]]), 0.0)
                xs.append(t)
            nc.gpsimd.tensor_copy(
                _ap(xs[2], 2, [[WP, BLC], [1, W]]),
                _ap(xb, 0, [[W, BLC], [1, W]]),
            )
            for dyi in (0, 1, 3, 4):
                pst = psx.tile([128, BLC * W], F32, name="pshift")
                nc.tensor.matmul(
                    pst[:, 0:512], sid[:, dyi * 128 : (dyi + 1) * 128],
                    xb[:, 0:512], start=True, stop=True,
                )
                nc.tensor.matmul(
                    pst[:, 512:1024], sid[:, dyi * 128 : (dyi + 1) * 128],
                    xb[:, 512:1024], start=True, stop=True,
                )
                nc.scalar.copy(
                    _ap(xs[dyi], 2, [[WP, BLC], [1, W]]),
                    _ap(pst, 0, [[W, BLC], [1, W]]),
                )

            # ---- per-channel main loop -----------------------------------
            # numden[c] accumulates [num | den]; tap j of dy-group mdy is the
            # two-chunk AP [m_j (256) | w_j (256)] (N=512 = one PSUM bank).
            # mdy layout: 5 m planes contiguous, then 5 w planes contiguous,
            # so the w- and m-products are one 1280-elem DVE instr each.
            outs = []
            nds = []
            rdens = []
            for c in range(C):
                nd = ps.tile([128, 2 * BW], F32, name=f"numden{c}")
                nds.append(nd)

                if c == 0:
                    l = l0
                else:
                    l = lp.tile([128, KK * BW], F32, name="l")
                    load_l(nc.sync, l, c, 0, 10)
                    load_l(nc.sync, l, c, 20, 5)
                    load_l(nc.gpsimd, l, c, 10, 10)
                # store channel c-2 now: its o2 is long finished, so the
                # semaphore wait never stalls the sync load queue.
                if c >= 2:
                    nc.sync.dma_start(
                        out=bass.AP(
                            yl, (c - 2) * HWr, [[W, H], [C * HWr, BL], [1, W]]
                        ),
                        in_=outs[c - 2][:],
                    )

                lexp = ep.tile([128, KK * BW], MM_DT, name="lexp")
                ksl = tuple((5 * dy, 5) for dy in range(5)) if c == 0 else KSPLIT
                for k0, nk in ksl:
                    nc.scalar.activation(
                        lexp[:, k0 * BW : (k0 + nk) * BW],
                        l[:, k0 * BW : (k0 + nk) * BW], AF.Exp,
                    )
                # deferred epilogue ACT half for the previous channel: these
                # sit behind this channel's exps so their numden wait is
                # already satisfied when reached.
                if c >= 1:
                    rdens.append(_epi_act(nc, op, nds[c - 1]))

                mdys = []
                for dy in range(5):
                    mdy = mp.tile([128, 2 * DB], MM_DT, name="mdy")
                    mdys.append(mdy)
                    # w block = s25 * lexp (one contiguous 1280-elem
                    # instr); the dy2 group runs on gpsimd to offload the
                    # vector engine (the kernel's critical resource)
                    weng = nc.gpsimd if dy == 2 else nc.vector
                    weng.tensor_tensor(
                        _ap(mdy, DB, [[1, DB]]),
                        _ap(s25, dy * DB, [[1, DB]]),
                        _ap(lexp, dy * DB, [[1, DB]]),
                        ALU.mult,
                    )
                    # m block = w block * xs window reads (taps j=0..4 read
                    # xs at column offset j)
                    nc.vector.tensor_tensor(
                        _ap(mdy, 0, [[1, DB]]),
                        _ap(mdy, DB, [[1, DB]]),
                        _ap(xs[dy], c * WP, [[1, 5], [C * WP, BL], [1, W]]),
                        ALU.mult,
                    )
                    # dy0..3 accumulate as one 20-matmul block (long PE run
                    # ramps the p-state); dy4's group closes the
                    # accumulation right after its products land.
                    if dy == 3:
                        for dyb in range(4):
                            for j in range(5):
                                nc.tensor.matmul(
                                    nd[:], idt,
                                    _ap(mdys[dyb], j * BW, [[DB, 2], [1, BW]]),
                                    start=(dyb == 0 and j == 0), stop=False,
                                )
                    elif dy == 4:
                        for j in range(5):
                            nc.tensor.matmul(
                                nd[:], idt,
                                _ap(mdy, j * BW, [[DB, 2], [1, BW]]),
                                start=False, stop=(j == 4),
                            )
                # deferred epilogue DVE half for the previous channel: sits
                # behind this channel's tap products, so its rden/numden
                # waits are satisfied when reached.
                if c >= 1:
                    _epi_dve(nc, op, outs, rdens[c - 1], nds[c - 1], xf, c - 1)

            rdens.append(_epi_act(nc, op, nds[C - 1]))
            _epi_dve(nc, op, outs, rdens[C - 1], nds[C - 1], xf, C - 1)
            for c in (C - 2, C - 1):
                nc.sync.dma_start(
                    out=bass.AP(yl, c * HWr, [[W, H], [C * HWr, BL], [1, W]]),
                    in_=outs[c][:],
                )

    _split_wide_waits(nc)
    return nc


def _epi_act(nc, op, nd):
    """1/den = exp(-ln(den)) on the ACT engine."""
    lden = op.tile([128, BW], F32, name="lden")
    nc.scalar.activation(lden[:], nd[:, BW : 2 * BW], AF.Ln)
    rden = op.tile([128, BW], F32, name="rden")
    nc.scalar.activation(rden[:], lden[:], AF.Exp, scale=-1.0)
    return rden


def _epi_dve(nc, op, outs, rden, nd, xf, c):
    """out_c = num * (1/den) + x on the vector engine."""
    o1 = op.tile([128, BW], F32, name="o1")
    nc.vector.scalar_tensor_tensor(
        o1[:], nd[:, 0:BW], 1.0, rden[:], ALU.bypass, ALU.mult
    )
    o2 = op.tile([128, BW], F32, name="o2")
    nc.vector.tensor_tensor(
        o2[:], o1[:], _ap(xf, c * W, [[C * W, BL], [1, W]]), ALU.add
    )
    outs.append(o2)


_NC_CACHE = None


def _get_nc():
    global _NC_CACHE
    if _NC_CACHE is None:
        _NC_CACHE = _build()
    return _NC_CACHE


def _make_in_maps(x, defocus_map, unet_out, alpha):
    x = np.ascontiguousarray(x, dtype=np.float32)
    alpha_s = np.float32(np.asarray(alpha).reshape(-1)[0])
    adf = np.ascontiguousarray(alpha_s * defocus_map, dtype=np.float32)
    unet_out = np.ascontiguousarray(unet_out, dtype=np.float32)
    in_maps = []
    for core in range(N_CORES):
        s = slice(core * BL, (core + 1) * BL)
        in_maps.append(
            {
                "x": x[s],
                "defocus": adf[s],
                "unet": unet_out[s],
            }
        )
    return in_maps


def run(x, defocus_map, unet_out, alpha, **spmd_kwargs):
    """Run the kernel; returns (output, BassKernelResults)."""
    nc = _get_nc()
    in_maps = _make_in_maps(x, defocus_map, unet_out, alpha)
    res = run_bass_kernel_spmd(nc, in_maps, list(range(N_CORES)), **spmd_kwargs)
    out = np.concatenate([res.results[i]["y"] for i in range(N_CORES)], axis=0)
    return out.astype(np.float32), res


def kernel(x, defocus_map, unet_out, alpha):
    return run(x, defocus_map, unet_out, alpha)[0]


# revision 12
# speedup vs baseline: 1.3493x; 1.3493x over previous
"""DefocusLKPN Trainium2 kernel.

Computes, per batch element (reference semantics):
    r      = clip(alpha * defocus + tanh(unet[:,100]), 0, 3)
    disk_k = sigmoid(5*(r - dist_k))            (25 taps, 6 distinct dists)
    w_ck   = exp(l_ck) * disk_k                 (l = unet[:, :100] logits)
    out_c  = sum_k w_ck * patch_ck / sum_k w_ck + x_c

The softmax normalizer and the EPS clamp of the reference cancel exactly
(center tap's disk mask is >= 0.5 for logits of this scale).  The alpha *
defocus product is folded into the defocus array on the host (alpha is a
learned scalar).

Sharding: pure data parallel, batch 16 -> 2 per core across 8 cores.

Per-core layout: partition dim = H (128); free dim packs (b, w) = 256 for
pixel planes and (k, b, w) for the 25-tap weight planes.  The 5x5 unfold is
realized as 5 row-shifted, column-padded copies of x in SBUF (vertical halo)
plus free-dim offsets (horizontal halo); the k-reduction runs on the tensor
engine as identity-matmul accumulation into PSUM (fp16 operands, f32
accumulation).

Performance notes (from HW traces):
  * SBUF->SBUF DMA row streams run at ~17 GB/s serialized on one queue --
    never used here.  The row-shifted x copies are built by shifted-identity
    matmuls into PSUM (tensor engine; zero-fills edge rows) and copied back
    to padded SBUF fp16 tiles by the ACT engine in its idle window between
    the sigmoids and the first exp (gpsimd cannot touch PSUM).
  * DVE fp16 tensor_tensor runs at 0.52 ns/elem (2x mode) with a ~150 ns
    fixed cost per instruction, so the tap-weight products are emitted as
    one 1280-elem instruction per (c, dy) group: mdy tiles pack the 5 m
    planes contiguous then the 5 w planes contiguous; the accumulate matmul
    reads tap j as the two-chunk AP [m_j | w_j].
  * The 25-plane replicated disk mask s25 lets the w-product be a single
    contiguous instruction; built once on the DVE right after the sigmoids.
  * 1/den uses ACT Ln then Exp(scale=-1) (~1.0us/channel) instead of DVE
    InstReciprocal (1.75us/channel).  Epilogue ACT ops are issued one
    channel late (behind the next channel's exps) and epilogue DVE ops after
    the next channel's tap products, so neither ever stalls its engine
    queue; stores are issued on sync two channels late for the same reason.
  * DMA descriptor generation runs at ~1.5 ns per 512B row, so one queue
    tops out near the ~286 GB/s aggregate engine rate.  Loads are split
    {0-9}+{20-24} on sync and {10-19} on gpsimd; the scalar engine issues
    no DMA so exp never stalls a load queue.  Radius-chain elementwise ops
    and the x cast run on gpsimd to keep the vector engine on tap products.
  * The accumulate matmuls are issued as one 20-matmul block (dy 0..3) plus
    the dy4 group, giving the PE long continuous runs to ramp out of the
    low p-state while keeping the post-last-byte tail short.
"""

import sys

sys.path.insert(0, "/opt/trn_rl_repo")

import numpy as np

import concourse.bass as bass
import concourse.mybir as mybir
from concourse.tile import TileContext
from concourse.bass_utils import run_bass_kernel_spmd

F32 = mybir.dt.float32
FP16 = mybir.dt.float16
AF = mybir.ActivationFunctionType
ALU = mybir.AluOpType

MM_DT = FP16

N_CORES = 8
B, C, H, W = 16, 4, 128, 128
BL = B // N_CORES            # 2 batch elements per core
BLC = BL * C                 # 8 (b, c) blocks
KK = 25
BW = BL * W                  # 256: (b, w) free block
WP = W + 4                   # 132: padded width per (b, c) block
DB = 5 * BW                  # 1280: one dy-group block (5 planes)

# distinct tap distances; k = (dy+2)*5 + (dx+2)
DISTS = [0.0, 1.0, np.sqrt(2.0), 2.0, np.sqrt(5.0), np.sqrt(8.0)]
# (dist_index, base_k, [(step, count), (step2, count2)]): tap sets sharing
# that dist, {base + i*s1 + j*s2}.
GROUPS = [
    (0, 12, []),                    # dist 0:      {12}
    (1, 7, [(6, 2), (4, 2)]),       # dist 1:      {7, 11, 13, 17}
    (2, 6, [(10, 2), (2, 2)]),      # dist sqrt2:  {6, 8, 16, 18}
    (3, 2, [(12, 2), (8, 2)]),      # dist 2:      {2, 10, 14, 22}
    (4, 5, [(10, 2), (4, 2)]),      # dist sqrt5:  {5, 9, 15, 19}
    (4, 1, [(20, 2), (2, 2)]),      # dist sqrt5:  {1, 3, 21, 23}
    (5, 0, [(20, 2), (4, 2)]),      # dist sqrt8:  {0, 4, 20, 24}
]

# l-load / exp k-range split: (k0, nk) per slice
KSPLIT = ((0, 10), (10, 10), (20, 5))


def _split_wide_waits(nc, max_waits=1):
    """The walrus build here accepts at most one semaphore wait per
    instruction; move extra waits onto preceding Drains on the same engine."""
    n = 0
    for func in nc.m.functions:
        for bb in func.blocks:
            out = []
            changed = False
            for ins in bb.instructions:
                si = ins.sync_info
                if si is not None and si.on_wait and len(si.on_wait) > max_waits:
                    waits = list(si.on_wait)
                    keep, rest = waits[:max_waits], waits[max_waits:]
                    for i in range(0, len(rest), max_waits):
                        n += 1
                        out.append(
                            mybir.InstDrain(
                                name=f"splitwait-{n}",
                                opcode="Drain",
                                engine=ins.engine,
                                sync_info=mybir.SyncInfo(
                                    on_wait=list(rest[i : i + max_waits]),
                                    on_update=[],
                                ),
                            )
                        )
                    si.on_wait = keep
                    changed = True
                out.append(ins)
            if changed:
                bb.instructions = out
    return n


def _ap(t, extra_off, dims):
    """AP over tile `t` keeping its partition dim, with free dims
    [[step, count], ...] in elements and an extra element offset."""
    return bass.AP(t.tensor, t.offset + extra_off, [list(t.ap[0])] + [list(d) for d in dims])


def _build():
    nc = bass.Bass("TRN2", num_devices=N_CORES)

    xl = nc.dram_tensor("x", [BL, C, H, W], F32, kind="ExternalInput")
    dfl = nc.dram_tensor("defocus", [BL, 1, H, W], F32, kind="ExternalInput")
    ul = nc.dram_tensor("unet", [BL, 4 * KK + 1, H, W], F32, kind="ExternalInput")
    yl = nc.dram_tensor("y", [BL, C, H, W], F32, kind="ExternalOutput")

    # 5 row-shift matrices S_dyi (dyi=0..4 <-> Dy=dyi-2); S.T @ x gives
    # x(i+Dy) with zero fill at the out-of-range edge rows.  Block dyi=2 is
    # the plain identity, reused as the accumulate-matmul stationary.
    s_np = np.zeros((128, 5 * 128), dtype=mybir.dt.np(MM_DT))
    for dyi in range(5):
        s_np[:, dyi * 128 : (dyi + 1) * 128] = np.eye(128, k=2 - dyi)
    sid_dram = nc.inline_tensor(s_np, name="sident")

    UCH = ul.shape[1]          # 101
    HWr = H * W                # plane stride in DRAM

    def load_l(eng, l, c, k0, nk):
        for b in range(BL):
            eng.dma_start(
                out=_ap(l, k0 * BW + b * W, [[BW, nk], [1, W]]),
                in_=bass.AP(
                    ul, (c * KK + k0 + b * UCH) * HWr,
                    [[W, H], [HWr, nk], [1, W]],
                ),
            )

    with TileContext(nc) as tc:
        with (
            tc.tile_pool(name="fix", bufs=1) as fix,
            tc.tile_pool(name="lp", bufs=3) as lp,
            tc.tile_pool(name="ep", bufs=2) as ep,
            tc.tile_pool(name="mp", bufs=6) as mp,
            tc.tile_pool(name="op", bufs=3) as op,
            tc.tile_pool(name="ps", bufs=1, space="PSUM") as ps,
            tc.tile_pool(name="psx", bufs=2, space="PSUM") as psx,
        ):
            # ---- prologue loads (sync queue, radius chain first) ---------
            df = fix.tile([128, BW], F32)
            nc.sync.dma_start(
                out=df[:],
                in_=bass.AP(dfl, 0, [[W, H], [HWr, BL], [1, W]]),
            )
            u100 = fix.tile([128, BW], F32)
            nc.sync.dma_start(
                out=u100[:],
                in_=bass.AP(ul, 100 * HWr, [[W, H], [UCH * HWr, BL], [1, W]]),
            )
            # first channel, finest granularity: 5-plane dy-aligned slices
            # split across both queues so the first tap products start as
            # early as possible (pipeline fill).  gpsimd carries the first
            # slices plus xf/sid; sync the later slices.
            l0 = lp.tile([128, KK * BW], F32, name="l")
            xf = fix.tile([128, BLC * W], F32)
            sid = fix.tile([128, 5 * 128], MM_DT)
            idt = sid[:, 2 * 128 : 3 * 128]
            load_l(nc.gpsimd, l0, 0, 0, 5)
            nc.gpsimd.dma_start(
                out=xf[:],
                in_=bass.AP(xl, 0, [[W, H], [HWr, BLC], [1, W]]),
            )
            nc.gpsimd.dma_start(out=sid[:], in_=sid_dram[:])
            load_l(nc.gpsimd, l0, 0, 5, 5)
            load_l(nc.sync, l0, 0, 10, 5)
            load_l(nc.sync, l0, 0, 15, 5)
            load_l(nc.sync, l0, 0, 20, 5)

            # radius = clip(adf + tanh(u100), 0, 3), adf = alpha*defocus
            # (host-folded).  Elementwise ops on gpsimd: vector stays free.
            dtan = fix.tile([128, BW], F32)
            nc.scalar.activation(dtan[:], u100[:], AF.Tanh)
            r0 = fix.tile([128, BW], F32)
            nc.vector.tensor_tensor(r0[:], df[:], dtan[:], ALU.add)
            rr = fix.tile([128, BW], F32)
            nc.vector.tensor_scalar(rr[:], r0[:], 0.0, 3.0, ALU.max, ALU.min)

            # s6[d] = sigmoid(5*r - 5*dist_d)   (6 planes, shared by all c)
            bt = fix.tile([128, 6], F32)
            for d in range(6):
                nc.gpsimd.memset(bt[:, d : d + 1], float(-5.0 * DISTS[d]))
            s6 = fix.tile([128, 6 * BW], MM_DT)
            for d in range(6):
                nc.scalar.activation(
                    s6[:, d * BW : (d + 1) * BW], rr[:], AF.Sigmoid,
                    bias=bt[:, d : d + 1], scale=5.0,
                )
            # s25[k] = s6[dist(k)]: replicated so the per-(c,dy) w-product is
            # a single contiguous DVE instruction.
            s25 = fix.tile([128, KK * BW], MM_DT)
            for d, base, steps in GROUPS:
                if steps:
                    (s1, c1), (s2, c2) = steps
                    odims = [[s1 * BW, c1], [s2 * BW, c2], [1, BW]]
                    idims = [[0, c1], [0, c2], [1, BW]]
                else:
                    odims = [[1, BW]]
                    idims = [[1, BW]]
                nc.vector.tensor_copy(
                    _ap(s25, base * BW, odims), _ap(s6, d * BW, idims)
                )

            # x cast to the matmul dtype (gpsimd; vector stays free)
            xb = fix.tile([128, BLC * W], MM_DT)
            nc.vector.tensor_copy(xb[:], xf[:])

            # xs[dyi]: row-shifted (by Dy=dyi-2), column-padded (pad 2) fp16
            # copies of x.  Row shift via S_dyi.T @ xb on the tensor engine
            # (PSUM, zero edge rows), copied to SBUF by the ACT engine in
            # its idle window before the first exp.
            xs = []
            for dyi in range(5):
                t = fix.tile([128, BLC * WP], MM_DT, name=f"xs{dyi}")
                nc.gpsimd.memset(_ap(t, 0, [[WP, BLC], [1, 2]]), 0.0)
                nc.gpsimd.memset(_ap(t, 2 + W, [[WP, BLC], [1, 2]]), 0.0)
                xs.append(t)
            nc.vector.tensor_copy(
                _ap(xs[2], 2, [[WP, BLC], [1, W]]),
                _ap(xb, 0, [[W, BLC], [1, W]]),
            )
            psts = {}
            for dyi in (0, 1, 3, 4):
                pst = psx.tile([128, BLC * W], F32, name="pshift")
                psts[dyi] = pst
                nc.tensor.matmul(
                    pst[:, 0:512], sid[:, dyi * 128 : (dyi + 1) * 128],
                    xb[:, 0:512], start=True, stop=True,
                )
                nc.tensor.matmul(
                    pst[:, 512:1024], sid[:, dyi * 128 : (dyi + 1) * 128],
                    xb[:, 512:1024], start=True, stop=True,
                )

            # ---- per-channel main loop -----------------------------------
            # numden[c] accumulates [num | den]; tap j of dy-group mdy is the
            # two-chunk AP [m_j (256) | w_j (256)] (N=512 = one PSUM bank).
            # mdy layout: 5 m planes contiguous, then 5 w planes contiguous,
            # so the w- and m-products are one 1280-elem DVE instr each.
            outs = []
            nds = []
            rdens = []
            for c in range(C):
                nd = ps.tile([128, 2 * BW], F32, name=f"numden{c}")
                nds.append(nd)

                if c == 0:
                    l = l0
                else:
                    l = lp.tile([128, KK * BW], F32, name="l")
                    load_l(nc.sync, l, c, 0, 10)
                    load_l(nc.sync, l, c, 20, 5)
                    load_l(nc.gpsimd, l, c, 10, 10)
                # store channel c-2 now: its o2 is long finished, so the
                # semaphore wait never stalls the sync load queue.
                if c >= 2:
                    nc.sync.dma_start(
                        out=bass.AP(
                            yl, (c - 2) * HWr, [[W, H], [C * HWr, BL], [1, W]]
                        ),
                        in_=outs[c - 2][:],
                    )

                lexp = ep.tile([128, KK * BW], MM_DT, name="lexp")
                ksl = tuple((5 * dy, 5) for dy in range(5)) if c == 0 else KSPLIT
                for si, (k0, nk) in enumerate(ksl):
                    nc.scalar.activation(
                        lexp[:, k0 * BW : (k0 + nk) * BW],
                        l[:, k0 * BW : (k0 + nk) * BW], AF.Exp,
                    )
                    if c == 0 and si in (0, 1, 3, 4):
                        nc.scalar.copy(
                            _ap(xs[si], 2, [[WP, BLC], [1, W]]),
                            _ap(psts[si], 0, [[W, BLC], [1, W]]),
                        )
                # deferred epilogue ACT half for the previous channel: these
                # sit behind this channel's exps so their numden wait is
                # already satisfied when reached.
                if c >= 1:
                    rdens.append(_epi_act(nc, op, nds[c - 1]))

                mdys = []
                for dy in range(5):
                    mdy = mp.tile([128, 2 * DB], MM_DT, name="mdy")
                    mdys.append(mdy)
                    # w block = s25 * lexp (one contiguous 1280-elem instr)
                    nc.vector.tensor_tensor(
                        _ap(mdy, DB, [[1, DB]]),
                        _ap(s25, dy * DB, [[1, DB]]),
                        _ap(lexp, dy * DB, [[1, DB]]),
                        ALU.mult,
                    )
                    # m block = w block * xs window reads (taps j=0..4 read
                    # xs at column offset j)
                    nc.vector.tensor_tensor(
                        _ap(mdy, 0, [[1, DB]]),
                        _ap(mdy, DB, [[1, DB]]),
                        _ap(xs[dy], c * WP, [[1, 5], [C * WP, BL], [1, W]]),
                        ALU.mult,
                    )
                    for j in range(5):
                        nc.tensor.matmul(
                            nd[:], idt,
                            _ap(mdy, j * BW, [[DB, 2], [1, BW]]),
                            start=(dy == 0 and j == 0), stop=(dy == 4 and j == 4),
                        )
                # deferred epilogue DVE half for the previous channel: sits
                # behind this channel's tap products, so its rden/numden
                # waits are satisfied when reached.
                if c >= 1:
                    _epi_dve(nc, op, outs, rdens[c - 1], nds[c - 1], xf, c - 1)

            rdens.append(_epi_act(nc, op, nds[C - 1]))
            _epi_dve(nc, op, outs, rdens[C - 1], nds[C - 1], xf, C - 1)
            for c in (C - 2, C - 1):
                nc.sync.dma_start(
                    out=bass.AP(yl, c * HWr, [[W, H], [C * HWr, BL], [1, W]]),
                    in_=outs[c][:],
                )

    _split_wide_waits(nc)
    return nc


def _epi_act(nc, op, nd):
    """1/den = exp(-ln(den)) on the ACT engine."""
    lden = op.tile([128, BW], F32, name="lden")
    nc.scalar.activation(lden[:], nd[:, BW : 2 * BW], AF.Ln)
    rden = op.tile([128, BW], F32, name="rden")
    nc.scalar.activation(rden[:], lden[:], AF.Exp, scale=-1.0)
    return rden


def _epi_dve(nc, op, outs, rden, nd, xf, c):
    """out_c = num * (1/den) + x on the vector engine."""
    o1 = op.tile([128, BW], F32, name="o1")
    nc.vector.scalar_tensor_tensor(
        o1[:], nd[:, 0:BW], 1.0, rden[:], ALU.bypass, ALU.mult
    )
    o2 = op.tile([128, BW], F32, name="o2")
    nc.vector.tensor_tensor(
        o2[:], o1[:], _ap(xf, c * W, [[C * W, BL], [1, W]]), ALU.add
    )
    outs.append(o2)


_NC_CACHE = None


def _get_nc():
    global _NC_CACHE
    if _NC_CACHE is None:
        _NC_CACHE = _build()
    return _NC_CACHE


def _make_in_maps(x, defocus_map, unet_out, alpha):
    x = np.ascontiguousarray(x, dtype=np.float32)
    alpha_s = np.float32(np.asarray(alpha).reshape(-1)[0])
    adf = np.ascontiguousarray(alpha_s * defocus_map, dtype=np.float32)
    unet_out = np.ascontiguousarray(unet_out, dtype=np.float32)
    in_maps = []
    for core in range(N_CORES):
        s = slice(core * BL, (core + 1) * BL)
        in_maps.append(
            {
                "x": x[s],
                "defocus": adf[s],
                "unet": unet_out[s],
            }
        )
    return in_maps


def run(x, defocus_map, unet_out, alpha, **spmd_kwargs):
    """Run the kernel; returns (output, BassKernelResults)."""
    nc = _get_nc()
    in_maps = _make_in_maps(x, defocus_map, unet_out, alpha)
    res = run_bass_kernel_spmd(nc, in_maps, list(range(N_CORES)), **spmd_kwargs)
    out = np.concatenate([res.results[i]["y"] for i in range(N_CORES)], axis=0)
    return out.astype(np.float32), res


def kernel(x, defocus_map, unet_out, alpha):
    return run(x, defocus_map, unet_out, alpha)[0]
